# revision 7
# baseline (speedup 1.0000x reference)
import sys
import numpy as np

sys.path.insert(0, '/opt/trn_rl_repo')

import concourse.bacc as bacc
import concourse.mybir as mybir
from concourse.bass_utils import run_bass_kernel_spmd
from concourse.tile import TileContext
from contextlib import ExitStack

f32 = mybir.dt.float32
f32r = mybir.dt.float32r
AF = mybir.ActivationFunctionType
ALU = mybir.AluOpType

D_MODEL = 1024
N_HEAD = 16
D_HEAD = 64
B = 4
T = 2048
N_CORES = 8
HPC = N_HEAD // 2        # 8 heads per core
HD = HPC * D_HEAD        # 512 head-dims per core
NTK = D_MODEL // 128     # 8 k-chunks over model dim
NTT = T // 128           # 16 T-tiles of 128
NJP = T // 1024          # 2 col-pairs of 1024

_cache = {}


def _build():
    nc = bacc.Bacc()
    xT = nc.declare_dram_parameter("xT", [D_MODEL, T], f32r, isOutput=False)
    wqkT = nc.declare_dram_parameter("wqkT", [D_MODEL, 2 * HD], f32r, isOutput=False)
    wvT = nc.declare_dram_parameter("wvT", [D_MODEL, HD], f32r, isOutput=False)
    wpT = nc.declare_dram_parameter("wpT", [HD, D_MODEL], f32r, isOutput=False)
    trimask = nc.declare_dram_parameter("trimask", [128, 128], f32r, isOutput=False)
    vones = nc.declare_dram_parameter("vones", [128, HPC], f32r, isOutput=False)
    outp = nc.declare_dram_parameter("out", [T, D_MODEL], f32, isOutput=True)

    with TileContext(nc) as tc, ExitStack() as outer:
        # persistent pools (whole kernel)
        qkp = outer.enter_context(tc.tile_pool(name="qk", bufs=1))
        vp = outer.enter_context(tc.tile_pool(name="v", bufs=1))
        ysbp = outer.enter_context(tc.tile_pool(name="ysb", bufs=1))
        wpp = outer.enter_context(tc.tile_pool(name="wp", bufs=1))
        smp = outer.enter_context(tc.tile_pool(name="small", bufs=1))

        mask = smp.tile([128, 128], f32r)
        nc.sync.dma_start(out=mask[:], in_=trimask[:, :])

        qk = [qkp.tile([128, T], f32r, tag=f"qk{m}", name=f"qk{m}") for m in range(8)]
        ysb = [ysbp.tile([128, T], f32r, tag=f"y{k}", name=f"ysb{k}") for k in range(4)]
        wp = []
        for k in range(HD // 128):
            t_ = wpp.tile([128, D_MODEL], f32r, tag=f"wp{k}", name=f"wpt{k}")
            nc.sync.dma_start(out=t_[:], in_=wpT[k * 128:(k + 1) * 128, :])
            wp.append(t_)
        vt = [None] * NTT

        # ---- S1: qkT[o,t], v_aug[t, 8*(64+1)]; x streamed in T-halves ----
        with ExitStack() as s1:
            xp = s1.enter_context(tc.tile_pool(name="x", bufs=1))
            wvp = s1.enter_context(tc.tile_pool(name="wv", bufs=1))
            wqp = s1.enter_context(tc.tile_pool(name="wqk", bufs=2))
            ps1 = s1.enter_context(tc.tile_pool(name="ps1", bufs=4, space="PSUM"))
            ps2 = s1.enter_context(tc.tile_pool(name="ps2", bufs=4, space="PSUM"))
            wv = []
            for k in range(NTK):
                t_ = wvp.tile([128, HD], f32r, tag=f"wv{k}", name=f"wv{k}")
                nc.sync.dma_start(out=t_[:], in_=wvT[k * 128:(k + 1) * 128, :])
                wv.append(t_)
            for th in range(2):
                hb = 1024 * th
                xt = []
                for k in range(NTK):
                    t_ = xp.tile([128, 1024], f32r, tag=f"x{k}", name=f"x{k}_{th}")
                    nc.sync.dma_start(out=t_[:], in_=xT[k * 128:(k + 1) * 128, hb:hb + 1024])
                    xt.append(t_)
                # S1a: q,k transposed
                for m in range(8):
                    wqm = []
                    for k in range(NTK):
                        t_ = wqp.tile([128, 128], f32r, tag=f"wm{k}", name=f"wm{k}_{m}_{th}")
                        nc.sync.dma_start(out=t_[:], in_=wqkT[k * 128:(k + 1) * 128,
                                                             m * 128:(m + 1) * 128])
                        wqm.append(t_)
                    for j in range(2):
                        ps = ps1.tile([128, 512], f32, tag="ps", name="ps1t")
                        for k in range(NTK):
                            nc.tensor.matmul(ps[:], wqm[k][:],
                                             xt[k][:, j * 512:(j + 1) * 512],
                                             start=(k == 0), stop=(k == NTK - 1))
                        nc.vector.tensor_copy(qk[m][:, hb + j * 512:hb + (j + 1) * 512], ps[:])
                # S1b: v natural + ones col
                for tl in range(8):
                    t = 8 * th + tl
                    va = vp.tile([128, HPC * 65], f32r, tag=f"v{t}", name=f"v{t}")
                    va3 = va[:].rearrange("p (h e) -> p h e", e=65)
                    nc.sync.dma_start(out=va3[:, :, 64], in_=vones[:, :])
                    ps = ps2.tile([128, HD], f32, tag="psv", name="ps2t")
                    for k in range(NTK):
                        nc.tensor.matmul(ps[:], xt[k][:, tl * 128:(tl + 1) * 128],
                                         wv[k][:, :],
                                         start=(k == 0), stop=(k == NTK - 1))
                    nc.vector.tensor_copy(
                        va3[:, :, 0:64],
                        ps[:].rearrange("p (h e) -> p h e", e=64))
                    vt[t] = va

        # ---- S2/S3: attention per head ----
        with ExitStack() as sa:
            pp = sa.enter_context(tc.tile_pool(name="p", bufs=4))
            bcp = sa.enter_context(tc.tile_pool(name="bc", bufs=2))
            psA = sa.enter_context(tc.tile_pool(name="psA", bufs=2, space="PSUM"))
            psY = sa.enter_context(tc.tile_pool(name="psY", bufs=2, space="PSUM"))
            for h in range(HPC):
                m, half = h // 2, h % 2
                rs = slice(64 * half, 64 * half + 64)
                qt, kt = qk[m], qk[4 + m]
                vsl = slice(65 * h, 65 * h + 65)
                for jp in range(NJP):
                    imax = min(NTT, 8 * jp + 8)
                    psy = psY.tile([65, 1024], f32, tag="psy", name="psyt")
                    for i in range(imax):
                        off = max(0, 128 * i - 1024 * jp)
                        psa = psA.tile([128, 1024], f32, tag="psa", name="psat")
                        for u in range(2):
                            s = max(off, 512 * u)
                            ce = 512 * u + 512
                            if s >= ce:
                                continue
                            nc.tensor.matmul(
                                psa[:, s:ce], kt[rs, i * 128:(i + 1) * 128],
                                qt[rs, 1024 * jp + s:1024 * jp + ce],
                                start=True, stop=True)
                        pt = pp.tile([128, 1024], f32r, tag="p", name="ptile")
                        nc.scalar.activation(pt[:, off:1024], psa[:, off:1024], AF.Exp)
                        if 8 * jp <= i:
                            nc.vector.tensor_tensor(
                                pt[:, off:off + 128], pt[:, off:off + 128],
                                mask[:], ALU.mult)
                        for u in range(2):
                            s = max(off, 512 * u)
                            ce = 512 * u + 512
                            if s >= ce:
                                continue
                            lasti = min(imax, 8 * jp + 4 * (u + 1)) - 1
                            nc.tensor.matmul(
                                psy[:, s:ce], vt[i][:, vsl], pt[:, s:ce],
                                start=(i == 0), stop=(i == lasti))
                    rc = bcp.tile([1, 1024], f32, tag="rc", name="rct")
                    nc.vector.reciprocal(rc[:], psy[64:65, :])
                    bc = bcp.tile([64, 1024], f32, tag="bc", name="bct")
                    nc.gpsimd.partition_broadcast(bc[:], rc[:])
                    nc.vector.tensor_tensor(
                        ysb[m][rs, 1024 * jp:1024 * (jp + 1)],
                        psy[0:64, :], bc[:], ALU.mult)

        # ---- S4: out[t, o] ----
        with ExitStack() as s4:
            ps4 = s4.enter_context(tc.tile_pool(name="ps4", bufs=4, space="PSUM"))
            ob = s4.enter_context(tc.tile_pool(name="ob", bufs=4))
            for t in range(NTT):
                for oc in range(D_MODEL // 512):
                    ps = ps4.tile([128, 512], f32, tag="ps", name="ps4t")
                    for k in range(4):
                        nc.tensor.matmul(
                            ps[:], ysb[k][:, t * 128:(t + 1) * 128],
                            wp[k][:, oc * 512:(oc + 1) * 512],
                            start=(k == 0), stop=(k == 3))
                    o_ = ob.tile([128, 512], f32, tag="o", name="obt")
                    nc.vector.tensor_copy(o_[:], ps[:])
                    nc.sync.dma_start(
                        out=outp[t * 128:(t + 1) * 128, oc * 512:(oc + 1) * 512],
                        in_=o_[:])

    nc.compile()
    return nc


def _prep_core_inputs(x, w_qkv, w_proj, c):
    b, g = c // 2, c % 2
    scale = np.float32(D_HEAD ** -0.5)
    wq = (w_qkv[g * HD:(g + 1) * HD] * scale).astype(np.float32)
    wk = w_qkv[D_MODEL + g * HD:D_MODEL + (g + 1) * HD]
    wv = w_qkv[2 * D_MODEL + g * HD:2 * D_MODEL + (g + 1) * HD]
    tri = np.triu(np.ones((128, 128), dtype=np.float32))
    return {
        "xT": np.ascontiguousarray(x[b].T),
        "wqkT": np.ascontiguousarray(np.concatenate([wq, wk], 0).T),
        "wvT": np.ascontiguousarray(wv.T),
        "wpT": np.ascontiguousarray(w_proj[:, g * HD:(g + 1) * HD].T),
        "trimask": tri,
        "vones": np.ones((128, HPC), dtype=np.float32),
    }


def kernel(x, w_qkv, w_proj):
    x = np.asarray(x)
    w_qkv = np.asarray(w_qkv)
    w_proj = np.asarray(w_proj)
    if "nc" not in _cache:
        _cache["nc"] = _build()
    nc = _cache["nc"]
    in_maps = [_prep_core_inputs(x, w_qkv, w_proj, c) for c in range(N_CORES)]
    res = run_bass_kernel_spmd(nc, in_maps, core_ids=list(range(N_CORES)))
    outs = [res.results[c]["out"] for c in range(N_CORES)]
    return np.stack([outs[2 * b] + outs[2 * b + 1] for b in range(B)], 0)


# revision 13
# speedup vs baseline: 1.1882x; 1.1882x over previous
import sys
import numpy as np

sys.path.insert(0, '/opt/trn_rl_repo')

import concourse.bacc as bacc
import concourse.mybir as mybir
from concourse.bass_utils import run_bass_kernel_spmd
from concourse.tile import TileContext
from contextlib import ExitStack

f32 = mybir.dt.float32
f32r = mybir.dt.float32r
AF = mybir.ActivationFunctionType
ALU = mybir.AluOpType

D_MODEL = 1024
N_HEAD = 16
D_HEAD = 64
B = 4
T = 2048
N_CORES = 8
HPC = N_HEAD // 2        # 8 heads per core
HD = HPC * D_HEAD        # 512 head-dims per core
NTK = D_MODEL // 128     # 8 k-chunks over model dim
NTT = T // 128           # 16 T-tiles of 128
NJP = T // 1024          # 2 col-pairs of 1024

_cache = {}


def _build():
    nc = bacc.Bacc()
    xT = nc.declare_dram_parameter("xT", [D_MODEL, T], f32r, isOutput=False)
    wqkT = nc.declare_dram_parameter("wqkT", [D_MODEL, 2 * HD], f32r, isOutput=False)
    wvT = nc.declare_dram_parameter("wvT", [D_MODEL, HD], f32r, isOutput=False)
    wpT = nc.declare_dram_parameter("wpT", [HD, D_MODEL], f32r, isOutput=False)
    trimask = nc.declare_dram_parameter("trimask", [128, 128], f32r, isOutput=False)
    vones = nc.declare_dram_parameter("vones", [128, HPC], f32r, isOutput=False)
    outp = nc.declare_dram_parameter("out", [T, D_MODEL], f32, isOutput=True)

    with TileContext(nc) as tc, ExitStack() as outer:
        # persistent pools (whole kernel)
        qkp = outer.enter_context(tc.tile_pool(name="qk", bufs=1))
        vp = outer.enter_context(tc.tile_pool(name="v", bufs=1))
        ysbp = outer.enter_context(tc.tile_pool(name="ysb", bufs=1))
        wpp = outer.enter_context(tc.tile_pool(name="wp", bufs=1))
        smp = outer.enter_context(tc.tile_pool(name="small", bufs=1))

        mask = smp.tile([128, 128], f32r)
        nc.sync.dma_start(out=mask[:], in_=trimask[:, :])
        # warm up the GPSIMD custom-op library load (~70us) during S1
        warm = smp.tile([2, 128], f32r)
        nc.gpsimd.partition_broadcast(warm[:], mask[0:1, :])

        qk = [qkp.tile([128, T], f32r, tag=f"qk{m}", name=f"qk{m}") for m in range(8)]
        ysb = [ysbp.tile([128, T], f32r, tag=f"y{k}", name=f"ysb{k}") for k in range(4)]
        wp = []
        for k in range(HD // 128):
            t_ = wpp.tile([128, D_MODEL], f32r, tag=f"wp{k}", name=f"wpt{k}")
            nc.sync.dma_start(out=t_[:], in_=wpT[k * 128:(k + 1) * 128, :])
            wp.append(t_)
        vt = [None] * NTT

        # ---- S1: qkT[o,t], v_aug[t, 8*(64+1)]; x streamed in T-halves ----
        with ExitStack() as s1:
            xp = s1.enter_context(tc.tile_pool(name="x", bufs=1))
            wvp = s1.enter_context(tc.tile_pool(name="wv", bufs=1))
            wqp = s1.enter_context(tc.tile_pool(name="wqk", bufs=2))
            ps1 = s1.enter_context(tc.tile_pool(name="ps1", bufs=4, space="PSUM"))
            ps2 = s1.enter_context(tc.tile_pool(name="ps2", bufs=4, space="PSUM"))
            wv = []
            for k in range(NTK):
                t_ = wvp.tile([128, HD], f32r, tag=f"wv{k}", name=f"wv{k}")
                nc.sync.dma_start(out=t_[:], in_=wvT[k * 128:(k + 1) * 128, :])
                wv.append(t_)
            for th in range(2):
                hb = 1024 * th
                xt = []
                for k in range(NTK):
                    t_ = xp.tile([128, 1024], f32r, tag=f"x{k}", name=f"x{k}_{th}")
                    nc.sync.dma_start(out=t_[:], in_=xT[k * 128:(k + 1) * 128, hb:hb + 1024])
                    xt.append(t_)
                # S1a: q,k transposed
                for m in range(8):
                    wqm = []
                    for k in range(NTK):
                        t_ = wqp.tile([128, 128], f32r, tag=f"wm{k}", name=f"wm{k}_{m}_{th}")
                        nc.sync.dma_start(out=t_[:], in_=wqkT[k * 128:(k + 1) * 128,
                                                             m * 128:(m + 1) * 128])
                        wqm.append(t_)
                    for j in range(2):
                        ps = ps1.tile([128, 512], f32, tag="ps", name="ps1t")
                        for k in range(NTK):
                            nc.tensor.matmul(ps[:], wqm[k][:],
                                             xt[k][:, j * 512:(j + 1) * 512],
                                             start=(k == 0), stop=(k == NTK - 1))
                        nc.vector.tensor_copy(qk[m][:, hb + j * 512:hb + (j + 1) * 512], ps[:])
                # S1b: v natural + ones col
                for tl in range(8):
                    t = 8 * th + tl
                    va = vp.tile([128, HPC * 65], f32r, tag=f"v{t}", name=f"v{t}")
                    va3 = va[:].rearrange("p (h e) -> p h e", e=65)
                    nc.sync.dma_start(out=va3[:, :, 64], in_=vones[:, :])
                    ps = ps2.tile([128, HD], f32, tag="psv", name="ps2t")
                    for k in range(NTK):
                        nc.tensor.matmul(ps[:], xt[k][:, tl * 128:(tl + 1) * 128],
                                         wv[k][:, :],
                                         start=(k == 0), stop=(k == NTK - 1))
                    nc.vector.tensor_copy(
                        va3[:, :, 0:64],
                        ps[:].rearrange("p (h e) -> p h e", e=64))
                    vt[t] = va

        # ---- S2/S3: attention per head ----
        with ExitStack() as sa:
            pp = sa.enter_context(tc.tile_pool(name="p", bufs=4))
            bcp = sa.enter_context(tc.tile_pool(name="bc", bufs=2))
            drp = sa.enter_context(tc.tile_pool(name="dr", bufs=2, space="DRAM"))
            psA = sa.enter_context(tc.tile_pool(name="psA", bufs=2, space="PSUM"))
            psY = sa.enter_context(tc.tile_pool(name="psY", bufs=2, space="PSUM"))
            for h in range(HPC):
                m, half = h // 2, h % 2
                rs = slice(64 * half, 64 * half + 64)
                qt, kt = qk[m], qk[4 + m]
                vsl = slice(65 * h, 65 * h + 65)
                for jp in range(NJP):
                    imax = min(NTT, 8 * jp + 8)
                    psy = psY.tile([65, 1024], f32, tag="psy", name="psyt")
                    psas = {}

                    def emit_qk(i, jp=jp, psas=psas, kt=kt, qt=qt, rs=rs):
                        off = max(0, 128 * i - 1024 * jp)
                        psa = psA.tile([128, 1024], f32, tag="psa", name="psat")
                        for u in range(2):
                            s = max(off, 512 * u)
                            ce = 512 * u + 512
                            if s >= ce:
                                continue
                            nc.tensor.matmul(
                                psa[:, s:ce], kt[rs, i * 128:(i + 1) * 128],
                                qt[rs, 1024 * jp + s:1024 * jp + ce],
                                start=True, stop=True)
                        psas[i] = psa

                    emit_qk(0)
                    for i in range(imax):
                        off = max(0, 128 * i - 1024 * jp)
                        if i + 1 < imax:
                            emit_qk(i + 1)
                        psa = psas.pop(i)
                        pt = pp.tile([128, 1024], f32r, tag="p", name="ptile")
                        nc.scalar.activation(pt[:, off:1024], psa[:, off:1024], AF.Exp)
                        if 8 * jp <= i:
                            nc.vector.tensor_tensor(
                                pt[:, off:off + 128], pt[:, off:off + 128],
                                mask[:], ALU.mult)
                        for u in range(2):
                            s = max(off, 512 * u)
                            ce = 512 * u + 512
                            if s >= ce:
                                continue
                            lasti = min(imax, 8 * jp + 4 * (u + 1)) - 1
                            nc.tensor.matmul(
                                psy[:, s:ce], vt[i][:, vsl], pt[:, s:ce],
                                start=(i == 0), stop=(i == lasti))
                    # reciprocal of the denominator row, reshaped across all
                    # 128 partitions (a [1,1024] DVE op runs on one lane)
                    drow = bcp.tile([1, 1024], f32, tag="drow", name="drowt")
                    nc.scalar.activation(drow[:], psy[64:65, :], AF.Copy)
                    dd = drp.tile([1024], f32, tag="dd", name="ddt")
                    nc.sync.dma_start(out=dd[:], in_=drow[0:1, :])
                    d8 = bcp.tile([128, 8], f32, tag="d8", name="d8t")
                    nc.sync.dma_start(
                        out=d8[:], in_=dd[:].rearrange("(a b) -> a b", a=128))
                    r8 = bcp.tile([128, 8], f32, tag="r8", name="r8t")
                    nc.vector.reciprocal(r8[:], d8[:])
                    rr = drp.tile([1024], f32, tag="rr", name="rrt")
                    nc.sync.dma_start(out=rr[:].rearrange("(a b) -> a b", a=128), in_=r8[:])
                    rc = bcp.tile([1, 1024], f32, tag="rc", name="rct")
                    nc.sync.dma_start(out=rc[0:1, :], in_=rr[:].rearrange("(q b) -> q b", q=1))
                    bc = bcp.tile([64, 1024], f32, tag="bc", name="bct")
                    nc.gpsimd.partition_broadcast(bc[:], rc[:])
                    nc.vector.tensor_tensor(
                        ysb[m][rs, 1024 * jp:1024 * (jp + 1)],
                        psy[0:64, :], bc[:], ALU.mult)

        # ---- S4: out[t, o] ----
        with ExitStack() as s4:
            ps4 = s4.enter_context(tc.tile_pool(name="ps4", bufs=4, space="PSUM"))
            ob = s4.enter_context(tc.tile_pool(name="ob", bufs=4))
            for t in range(NTT):
                for oc in range(D_MODEL // 512):
                    ps = ps4.tile([128, 512], f32, tag="ps", name="ps4t")
                    for k in range(4):
                        nc.tensor.matmul(
                            ps[:], ysb[k][:, t * 128:(t + 1) * 128],
                            wp[k][:, oc * 512:(oc + 1) * 512],
                            start=(k == 0), stop=(k == 3))
                    o_ = ob.tile([128, 512], f32, tag="o", name="obt")
                    nc.vector.tensor_copy(o_[:], ps[:])
                    nc.sync.dma_start(
                        out=outp[t * 128:(t + 1) * 128, oc * 512:(oc + 1) * 512],
                        in_=o_[:])

    nc.compile()
    return nc


def _prep_core_inputs(x, w_qkv, w_proj, c):
    b, g = c // 2, c % 2
    scale = np.float32(D_HEAD ** -0.5)
    wq = (w_qkv[g * HD:(g + 1) * HD] * scale).astype(np.float32)
    wk = w_qkv[D_MODEL + g * HD:D_MODEL + (g + 1) * HD]
    wv = w_qkv[2 * D_MODEL + g * HD:2 * D_MODEL + (g + 1) * HD]
    tri = np.triu(np.ones((128, 128), dtype=np.float32))
    return {
        "xT": np.ascontiguousarray(x[b].T),
        "wqkT": np.ascontiguousarray(np.concatenate([wq, wk], 0).T),
        "wvT": np.ascontiguousarray(wv.T),
        "wpT": np.ascontiguousarray(w_proj[:, g * HD:(g + 1) * HD].T),
        "trimask": tri,
        "vones": np.ones((128, HPC), dtype=np.float32),
    }


def kernel(x, w_qkv, w_proj):
    x = np.asarray(x)
    w_qkv = np.asarray(w_qkv)
    w_proj = np.asarray(w_proj)
    if "nc" not in _cache:
        _cache["nc"] = _build()
    nc = _cache["nc"]
    in_maps = [_prep_core_inputs(x, w_qkv, w_proj, c) for c in range(N_CORES)]
    res = run_bass_kernel_spmd(nc, in_maps, core_ids=list(range(N_CORES)))
    outs = [res.results[c]["out"] for c in range(N_CORES)]
    return np.stack([outs[2 * b] + outs[2 * b + 1] for b in range(B)], 0)


# revision 18
# speedup vs baseline: 1.3800x; 1.1615x over previous
import sys
import numpy as np

sys.path.insert(0, '/opt/trn_rl_repo')

import concourse.bacc as bacc
import concourse.mybir as mybir
from concourse.bass_utils import run_bass_kernel_spmd
from concourse.tile import TileContext
from contextlib import ExitStack

f32 = mybir.dt.float32
f32r = mybir.dt.float32r
AF = mybir.ActivationFunctionType
ALU = mybir.AluOpType

D_MODEL = 1024
N_HEAD = 16
D_HEAD = 64
B = 4
T = 2048
N_CORES = 8
HPC = N_HEAD // 2        # 8 heads per core
HD = HPC * D_HEAD        # 512 head-dims per core
NTK = D_MODEL // 128     # 8 k-chunks over model dim
NTT = T // 128           # 16 T-tiles of 128
NJP = T // 1024          # 2 col-pairs of 1024

_cache = {}


def _build():
    nc = bacc.Bacc()
    xT = nc.declare_dram_parameter("xT", [D_MODEL, T], f32r, isOutput=False)
    wqkT = nc.declare_dram_parameter("wqkT", [D_MODEL, 2 * HD], f32r, isOutput=False)
    wvT = nc.declare_dram_parameter("wvT", [D_MODEL, HD], f32r, isOutput=False)
    wpT = nc.declare_dram_parameter("wpT", [HD, D_MODEL], f32r, isOutput=False)
    trimask = nc.declare_dram_parameter("trimask", [128, 128], f32r, isOutput=False)
    vones = nc.declare_dram_parameter("vones", [128, HPC], f32r, isOutput=False)
    outp = nc.declare_dram_parameter("out", [T, D_MODEL], f32, isOutput=True)

    with TileContext(nc) as tc, ExitStack() as outer:
        # persistent pools (whole kernel)
        qkp = outer.enter_context(tc.tile_pool(name="qk", bufs=1))
        vp = outer.enter_context(tc.tile_pool(name="v", bufs=1))
        ysbp = outer.enter_context(tc.tile_pool(name="ysb", bufs=1))
        wpp = outer.enter_context(tc.tile_pool(name="wp", bufs=1))
        smp = outer.enter_context(tc.tile_pool(name="small", bufs=1))

        qk = [qkp.tile([128, T], f32r, tag=f"qk{m}", name=f"qk{m}") for m in range(8)]
        ysb = [ysbp.tile([128, T], f32r, tag=f"y{k}", name=f"ysb{k}") for k in range(4)]
        vt = [None] * NTT
        mask = smp.tile([128, 128], f32r)
        warm = smp.tile([2, 128], f32r)

        # ---- S1: qkT[o,t], v_aug[t, 8*(64+1)]; x streamed in T-halves ----
        with ExitStack() as s1:
            xp = s1.enter_context(tc.tile_pool(name="x", bufs=1))
            wvp = s1.enter_context(tc.tile_pool(name="wv", bufs=1))
            wqp = s1.enter_context(tc.tile_pool(name="wqk", bufs=2))
            ps1 = s1.enter_context(tc.tile_pool(name="ps1", bufs=4, space="PSUM"))
            ps2 = s1.enter_context(tc.tile_pool(name="ps2", bufs=4, space="PSUM"))
            # x half 0 first: the first S1a matmuls gate on it
            xt0 = []
            for k in range(NTK):
                t_ = xp.tile([128, 1024], f32r, tag=f"x{k}", name=f"x{k}_0")
                nc.sync.dma_start(out=t_[:], in_=xT[k * 128:(k + 1) * 128, 0:1024])
                xt0.append(t_)
            wv = []
            for k in range(NTK):
                t_ = wvp.tile([128, HD], f32r, tag=f"wv{k}", name=f"wv{k}")
                nc.sync.dma_start(out=t_[:], in_=wvT[k * 128:(k + 1) * 128, :])
                wv.append(t_)
            nc.sync.dma_start(out=mask[:], in_=trimask[:, :])
            # warm up the GPSIMD custom-op library load (~70us) during S1
            nc.gpsimd.partition_broadcast(warm[:], mask[0:1, :])
            for th in range(2):
                hb = 1024 * th
                if th == 0:
                    xt = xt0
                else:
                    xt = []
                    for k in range(NTK):
                        t_ = xp.tile([128, 1024], f32r, tag=f"x{k}", name=f"x{k}_{th}")
                        nc.sync.dma_start(out=t_[:],
                                          in_=xT[k * 128:(k + 1) * 128, hb:hb + 1024])
                        xt.append(t_)
                # S1a: q,k transposed
                for m in range(8):
                    wqm = []
                    for k in range(NTK):
                        t_ = wqp.tile([128, 128], f32r, tag=f"wm{k}", name=f"wm{k}_{m}_{th}")
                        nc.sync.dma_start(out=t_[:], in_=wqkT[k * 128:(k + 1) * 128,
                                                             m * 128:(m + 1) * 128])
                        wqm.append(t_)
                    for j in range(2):
                        ps = ps1.tile([128, 512], f32, tag="ps", name="ps1t")
                        for k in range(NTK):
                            nc.tensor.matmul(ps[:], wqm[k][:],
                                             xt[k][:, j * 512:(j + 1) * 512],
                                             start=(k == 0), stop=(k == NTK - 1))
                        nc.vector.tensor_copy(qk[m][:, hb + j * 512:hb + (j + 1) * 512], ps[:])
                # S1b: v natural + ones col
                for tl in range(8):
                    t = 8 * th + tl
                    va = vp.tile([128, HPC * 65], f32r, tag=f"v{t}", name=f"v{t}")
                    va3 = va[:].rearrange("p (h e) -> p h e", e=65)
                    nc.sync.dma_start(out=va3[:, :, 64], in_=vones[:, :])
                    ps = ps2.tile([128, HD], f32, tag="psv", name="ps2t")
                    for k in range(NTK):
                        nc.tensor.matmul(ps[:], xt[k][:, tl * 128:(tl + 1) * 128],
                                         wv[k][:, :],
                                         start=(k == 0), stop=(k == NTK - 1))
                    nc.vector.tensor_copy(
                        va3[:, :, 0:64],
                        ps[:].rearrange("p (h e) -> p h e", e=64))
                    vt[t] = va

        # wp is only read by S4; load it during attention
        wp = []
        for k in range(HD // 128):
            t_ = wpp.tile([128, D_MODEL], f32r, tag=f"wp{k}", name=f"wpt{k}")
            nc.sync.dma_start(out=t_[:], in_=wpT[k * 128:(k + 1) * 128, :])
            wp.append(t_)

        # ---- S2/S3: attention, head pairs packed via tile_position ----
        with ExitStack() as sa:
            pp = sa.enter_context(tc.tile_pool(name="p", bufs=6))
            ytp = sa.enter_context(tc.tile_pool(name="yt", bufs=2))
            bcp = sa.enter_context(tc.tile_pool(name="bc", bufs=2))
            drp = sa.enter_context(tc.tile_pool(name="dr", bufs=2, space="DRAM"))
            psA = sa.enter_context(tc.tile_pool(name="psA", bufs=2, space="PSUM"))
            psY = sa.enter_context(tc.tile_pool(name="psY", bufs=1, space="PSUM"))
            for m in range(4):
                hA, hB = 2 * m, 2 * m + 1
                qt, kt = qk[m], qk[4 + m]
                for jp in range(NJP):
                    imax = min(NTT, 8 * jp + 8)
                    psyA = psY.tile([65, 1024], f32, tag="psyA", name="psyAt")
                    psyB = psY.tile([65, 1024], f32, tag="psyB", name="psyBt")
                    psas = {}

                    def emit_qk(i, jp=jp, psas=psas, kt=kt, qt=qt):
                        off = max(0, 128 * i - 1024 * jp)
                        for u in range(2):
                            su = max(off - 512 * u, 0)
                            if su >= 512:
                                continue
                            n = 512 - su
                            tcol = 1024 * jp + 512 * u + su
                            psa = psA.tile([128, 1024], f32, tag="psa", name="psat")
                            nc.tensor.matmul(
                                psa[:, su:512], kt[0:64, i * 128:(i + 1) * 128],
                                qt[0:64, tcol:tcol + n],
                                start=True, stop=True, tile_position=(0, 0))
                            nc.tensor.matmul(
                                psa[:, 512 + su:1024], kt[64:128, i * 128:(i + 1) * 128],
                                qt[64:128, tcol:tcol + n],
                                start=True, stop=True, tile_position=(64, 0))
                            psas[(i, u)] = psa

                    emit_qk(0)
                    for i in range(imax):
                        off = max(0, 128 * i - 1024 * jp)
                        if i + 1 < imax:
                            emit_qk(i + 1)
                        for u in range(2):
                            su = max(off - 512 * u, 0)
                            if su >= 512:
                                continue
                            psa = psas.pop((i, u))
                            pt = pp.tile([128, 1024], f32r, tag="p", name="ptile")
                            p3i = psa[:].rearrange("p (g c) -> p g c", g=2)
                            p3o = pt[:].rearrange("p (g c) -> p g c", g=2)
                            nc.scalar.activation(p3o[:, :, su:512], p3i[:, :, su:512],
                                                 AF.Exp)
                            if 8 * jp <= i and 512 * u <= off < 512 * (u + 1):
                                nc.vector.tensor_tensor(
                                    pt[:, su:su + 128], pt[:, su:su + 128],
                                    mask[:], ALU.mult)
                                nc.vector.tensor_tensor(
                                    pt[:, 512 + su:512 + su + 128],
                                    pt[:, 512 + su:512 + su + 128],
                                    mask[:], ALU.mult)
                            lasti = min(imax, 8 * jp + 4 * (u + 1)) - 1
                            yc = slice(512 * u + su, 512 * u + 512)
                            nc.tensor.matmul(
                                psyA[:, yc], vt[i][:, 65 * hA:65 * hA + 65],
                                pt[:, su:512], start=(i == 0), stop=(i == lasti))
                            nc.tensor.matmul(
                                psyB[:, yc], vt[i][:, 65 * hB:65 * hB + 65],
                                pt[:, 512 + su:1024], start=(i == 0), stop=(i == lasti))
                    for h, psy in ((hA, psyA), (hB, psyB)):
                        rs = slice(64 * (h % 2), 64 * (h % 2) + 64)
                        # evict unnormalized y early so the PSUM bank frees
                        yt = ytp.tile([64, 1024], f32, tag="yt", name="ytt")
                        nc.vector.tensor_copy(yt[:], psy[0:64, :])
                        # reciprocal of denominator row, reshaped across 128
                        # partitions (a [1,1024] DVE op runs on one lane)
                        drow = bcp.tile([1, 1024], f32, tag="drow", name="drowt")
                        nc.scalar.activation(drow[:], psy[64:65, :], AF.Copy)
                        dd = drp.tile([1024], f32, tag="dd", name="ddt")
                        nc.sync.dma_start(out=dd[:], in_=drow[0:1, :])
                        d8 = bcp.tile([128, 8], f32, tag="d8", name="d8t")
                        nc.sync.dma_start(
                            out=d8[:], in_=dd[:].rearrange("(a b) -> a b", a=128))
                        r8 = bcp.tile([128, 8], f32, tag="r8", name="r8t")
                        nc.vector.reciprocal(r8[:], d8[:])
                        rr = drp.tile([1024], f32, tag="rr", name="rrt")
                        nc.sync.dma_start(out=rr[:].rearrange("(a b) -> a b", a=128),
                                          in_=r8[:])
                        rc = bcp.tile([1, 1024], f32, tag="rc", name="rct")
                        nc.sync.dma_start(out=rc[0:1, :],
                                          in_=rr[:].rearrange("(q b) -> q b", q=1))
                        bc = bcp.tile([64, 1024], f32, tag="bc", name="bct")
                        nc.gpsimd.partition_broadcast(bc[:], rc[:])
                        nc.vector.tensor_tensor(
                            ysb[m][rs, 1024 * jp:1024 * (jp + 1)],
                            yt[:], bc[:], ALU.mult)

        # ---- S4: out[t, o] ----
        with ExitStack() as s4:
            ps4 = s4.enter_context(tc.tile_pool(name="ps4", bufs=4, space="PSUM"))
            ob = s4.enter_context(tc.tile_pool(name="ob", bufs=4))
            for t in range(NTT):
                for oc in range(D_MODEL // 512):
                    ps = ps4.tile([128, 512], f32, tag="ps", name="ps4t")
                    for k in range(4):
                        nc.tensor.matmul(
                            ps[:], ysb[k][:, t * 128:(t + 1) * 128],
                            wp[k][:, oc * 512:(oc + 1) * 512],
                            start=(k == 0), stop=(k == 3))
                    o_ = ob.tile([128, 512], f32, tag="o", name="obt")
                    nc.vector.tensor_copy(o_[:], ps[:])
                    nc.sync.dma_start(
                        out=outp[t * 128:(t + 1) * 128, oc * 512:(oc + 1) * 512],
                        in_=o_[:])

    nc.compile()
    return nc


def _prep_core_inputs(x, w_qkv, w_proj, c):
    b, g = c // 2, c % 2
    scale = np.float32(D_HEAD ** -0.5)
    wq = (w_qkv[g * HD:(g + 1) * HD] * scale).astype(np.float32)
    wk = w_qkv[D_MODEL + g * HD:D_MODEL + (g + 1) * HD]
    wv = w_qkv[2 * D_MODEL + g * HD:2 * D_MODEL + (g + 1) * HD]
    tri = np.triu(np.ones((128, 128), dtype=np.float32))
    return {
        "xT": np.ascontiguousarray(x[b].T),
        "wqkT": np.ascontiguousarray(np.concatenate([wq, wk], 0).T),
        "wvT": np.ascontiguousarray(wv.T),
        "wpT": np.ascontiguousarray(w_proj[:, g * HD:(g + 1) * HD].T),
        "trimask": tri,
        "vones": np.ones((128, HPC), dtype=np.float32),
    }


def kernel(x, w_qkv, w_proj):
    x = np.asarray(x)
    w_qkv = np.asarray(w_qkv)
    w_proj = np.asarray(w_proj)
    if "nc" not in _cache:
        _cache["nc"] = _build()
    nc = _cache["nc"]
    in_maps = [_prep_core_inputs(x, w_qkv, w_proj, c) for c in range(N_CORES)]
    res = run_bass_kernel_spmd(nc, in_maps, core_ids=list(range(N_CORES)))
    outs = [res.results[c]["out"] for c in range(N_CORES)]
    return np.stack([outs[2 * b] + outs[2 * b + 1] for b in range(B)], 0)


# revision 21
# speedup vs baseline: 1.3909x; 1.0079x over previous
import sys
import numpy as np

sys.path.insert(0, '/opt/trn_rl_repo')

import concourse.bacc as bacc
import concourse.mybir as mybir
from concourse.bass_utils import run_bass_kernel_spmd
from concourse.tile import TileContext
from contextlib import ExitStack

f32 = mybir.dt.float32
f32r = mybir.dt.float32r
AF = mybir.ActivationFunctionType
ALU = mybir.AluOpType

D_MODEL = 1024
N_HEAD = 16
D_HEAD = 64
B = 4
T = 2048
N_CORES = 8
HPC = N_HEAD // 2        # 8 heads per core
HD = HPC * D_HEAD        # 512 head-dims per core
NTK = D_MODEL // 128     # 8 k-chunks over model dim
NTT = T // 128           # 16 T-tiles of 128
NJP = T // 1024          # 2 col-pairs of 1024

_cache = {}


def _build():
    nc = bacc.Bacc()
    xT = nc.declare_dram_parameter("xT", [D_MODEL, T], f32r, isOutput=False)
    wqkT = nc.declare_dram_parameter("wqkT", [D_MODEL, 2 * HD], f32r, isOutput=False)
    wvT = nc.declare_dram_parameter("wvT", [D_MODEL, HD], f32r, isOutput=False)
    wpT = nc.declare_dram_parameter("wpT", [HD, D_MODEL], f32r, isOutput=False)
    trimask = nc.declare_dram_parameter("trimask", [128, 128], f32r, isOutput=False)
    vones = nc.declare_dram_parameter("vones", [128, HPC], f32r, isOutput=False)
    outp = nc.declare_dram_parameter("out", [T, D_MODEL], f32, isOutput=True)

    with TileContext(nc) as tc, ExitStack() as outer:
        # persistent pools (whole kernel)
        qkp = outer.enter_context(tc.tile_pool(name="qk", bufs=1))
        vp = outer.enter_context(tc.tile_pool(name="v", bufs=1))
        smp = outer.enter_context(tc.tile_pool(name="small", bufs=1))

        qk = [qkp.tile([128, T], f32r, tag=f"qk{m}", name=f"qk{m}") for m in range(8)]
        vt = [None] * NTT
        mask = smp.tile([128, 128], f32r)
        warm = smp.tile([2, 128], f32r)

        # ---- S1: qkT[o,t], v_aug[t, 8*(64+1)]; x streamed in T-halves ----
        with ExitStack() as s1:
            xp = s1.enter_context(tc.tile_pool(name="x", bufs=2))
            wvp = s1.enter_context(tc.tile_pool(name="wv", bufs=1))
            wqp = s1.enter_context(tc.tile_pool(name="wqk", bufs=2))
            ps1 = s1.enter_context(tc.tile_pool(name="ps1", bufs=4, space="PSUM"))
            ps2 = s1.enter_context(tc.tile_pool(name="ps2", bufs=4, space="PSUM"))
            # x first: the first S1a matmuls gate on it; j-split halves so
            # the m=0,j=0 chain starts after ~2MB; ACT HWDGE queue keeps this
            # traffic off the sync queue
            xts = {0: [], 1: []}
            for th in range(2):
                for k in range(NTK):
                    t_ = xp.tile([128, 1024], f32r, tag=f"x{k}", name=f"x{k}_{th}")
                    xts[th].append(t_)
            for th in range(2):
                for jh in range(2):
                    for k in range(NTK):
                        nc.scalar.dma_start(
                            out=xts[th][k][:, jh * 512:(jh + 1) * 512],
                            in_=xT[k * 128:(k + 1) * 128,
                                   1024 * th + jh * 512:1024 * th + (jh + 1) * 512])
            wv = []
            for k in range(NTK):
                t_ = wvp.tile([128, HD], f32r, tag=f"wv{k}", name=f"wv{k}")
                nc.scalar.dma_start(out=t_[:], in_=wvT[k * 128:(k + 1) * 128, :])
                wv.append(t_)
            nc.sync.dma_start(out=mask[:], in_=trimask[:, :])
            # warm up the GPSIMD custom-op library load (~70us) during S1
            nc.gpsimd.partition_broadcast(warm[:], mask[0:1, :])
            for th in range(2):
                hb = 1024 * th
                xt = xts[th]
                # S1a: q,k transposed
                for m in range(8):
                    wqm = []
                    for k in range(NTK):
                        t_ = wqp.tile([128, 128], f32r, tag=f"wm{k}", name=f"wm{k}_{m}_{th}")
                        nc.sync.dma_start(out=t_[:], in_=wqkT[k * 128:(k + 1) * 128,
                                                             m * 128:(m + 1) * 128])
                        wqm.append(t_)
                    for j in range(2):
                        ps = ps1.tile([128, 512], f32, tag="ps", name="ps1t")
                        for k in range(NTK):
                            nc.tensor.matmul(ps[:], wqm[k][:],
                                             xt[k][:, j * 512:(j + 1) * 512],
                                             start=(k == 0), stop=(k == NTK - 1))
                        nc.vector.tensor_copy(qk[m][:, hb + j * 512:hb + (j + 1) * 512], ps[:])
                # S1b: v natural + ones col
                for tl in range(8):
                    t = 8 * th + tl
                    va = vp.tile([128, HPC * 65], f32r, tag=f"v{t}", name=f"v{t}")
                    va3 = va[:].rearrange("p (h e) -> p h e", e=65)
                    nc.sync.dma_start(out=va3[:, :, 64], in_=vones[:, :])
                    ps = ps2.tile([128, HD], f32, tag="psv", name="ps2t")
                    for k in range(NTK):
                        nc.tensor.matmul(ps[:], xt[k][:, tl * 128:(tl + 1) * 128],
                                         wv[k][:, :],
                                         start=(k == 0), stop=(k == NTK - 1))
                    nc.vector.tensor_copy(
                        va3[:, :, 0:64],
                        ps[:].rearrange("p (h e) -> p h e", e=64))
                    vt[t] = va

        # ---- S2/S3: attention, head pairs packed via tile_position ----
        with ExitStack() as sa:
            ysbp = sa.enter_context(tc.tile_pool(name="ysb", bufs=1))
            wpp = sa.enter_context(tc.tile_pool(name="wp", bufs=1))
            ysb = [ysbp.tile([128, T], f32r, tag=f"y{k}", name=f"ysb{k}") for k in range(4)]
            # wp is only read by S4; load it during attention
            wp = []
            for k in range(HD // 128):
                t_ = wpp.tile([128, D_MODEL], f32r, tag=f"wp{k}", name=f"wpt{k}")
                nc.sync.dma_start(out=t_[:], in_=wpT[k * 128:(k + 1) * 128, :])
                wp.append(t_)
            sab = sa.enter_context(ExitStack())
            pp = sab.enter_context(tc.tile_pool(name="p", bufs=6))
            ytp = sab.enter_context(tc.tile_pool(name="yt", bufs=2))
            bcp = sab.enter_context(tc.tile_pool(name="bc", bufs=2))
            drp = sab.enter_context(tc.tile_pool(name="dr", bufs=2, space="DRAM"))
            psA = sab.enter_context(tc.tile_pool(name="psA", bufs=2, space="PSUM"))
            psY = sab.enter_context(tc.tile_pool(name="psY", bufs=1, space="PSUM"))
            for m in range(4):
                hA, hB = 2 * m, 2 * m + 1
                qt, kt = qk[m], qk[4 + m]
                for jp in range(NJP):
                    imax = min(NTT, 8 * jp + 8)
                    psyA = psY.tile([65, 1024], f32, tag="psyA", name="psyAt")
                    psyB = psY.tile([65, 1024], f32, tag="psyB", name="psyBt")
                    psas = {}

                    def emit_qk(i, jp=jp, psas=psas, kt=kt, qt=qt):
                        off = max(0, 128 * i - 1024 * jp)
                        for u in range(2):
                            su = max(off - 512 * u, 0)
                            if su >= 512:
                                continue
                            n = 512 - su
                            tcol = 1024 * jp + 512 * u + su
                            psa = psA.tile([128, 1024], f32, tag="psa", name="psat")
                            nc.tensor.matmul(
                                psa[:, su:512], kt[0:64, i * 128:(i + 1) * 128],
                                qt[0:64, tcol:tcol + n],
                                start=True, stop=True, tile_position=(0, 0))
                            nc.tensor.matmul(
                                psa[:, 512 + su:1024], kt[64:128, i * 128:(i + 1) * 128],
                                qt[64:128, tcol:tcol + n],
                                start=True, stop=True, tile_position=(64, 0))
                            psas[(i, u)] = psa

                    emit_qk(0)
                    for i in range(imax):
                        off = max(0, 128 * i - 1024 * jp)
                        if i + 1 < imax:
                            emit_qk(i + 1)
                        for u in range(2):
                            su = max(off - 512 * u, 0)
                            if su >= 512:
                                continue
                            psa = psas.pop((i, u))
                            pt = pp.tile([128, 1024], f32r, tag="p", name="ptile")
                            p3i = psa[:].rearrange("p (g c) -> p g c", g=2)
                            p3o = pt[:].rearrange("p (g c) -> p g c", g=2)
                            nc.scalar.activation(p3o[:, :, su:512], p3i[:, :, su:512],
                                                 AF.Exp)
                            if 8 * jp <= i and 512 * u <= off < 512 * (u + 1):
                                nc.vector.tensor_tensor(
                                    pt[:, su:su + 128], pt[:, su:su + 128],
                                    mask[:], ALU.mult)
                                nc.vector.tensor_tensor(
                                    pt[:, 512 + su:512 + su + 128],
                                    pt[:, 512 + su:512 + su + 128],
                                    mask[:], ALU.mult)
                            lasti = min(imax, 8 * jp + 4 * (u + 1)) - 1
                            yc = slice(512 * u + su, 512 * u + 512)
                            nc.tensor.matmul(
                                psyA[:, yc], vt[i][:, 65 * hA:65 * hA + 65],
                                pt[:, su:512], start=(i == 0), stop=(i == lasti))
                            nc.tensor.matmul(
                                psyB[:, yc], vt[i][:, 65 * hB:65 * hB + 65],
                                pt[:, 512 + su:1024], start=(i == 0), stop=(i == lasti))
                    for h, psy in ((hA, psyA), (hB, psyB)):
                        rs = slice(64 * (h % 2), 64 * (h % 2) + 64)
                        # evict unnormalized y early so the PSUM bank frees
                        yt = ytp.tile([64, 1024], f32, tag="yt", name="ytt")
                        nc.vector.tensor_copy(yt[:], psy[0:64, :])
                        # reciprocal of denominator row, reshaped across 128
                        # partitions (a [1,1024] DVE op runs on one lane)
                        drow = bcp.tile([1, 1024], f32, tag="drow", name="drowt")
                        nc.scalar.activation(drow[:], psy[64:65, :], AF.Copy)
                        dd = drp.tile([1024], f32, tag="dd", name="ddt")
                        nc.sync.dma_start(out=dd[:], in_=drow[0:1, :])
                        d8 = bcp.tile([128, 8], f32, tag="d8", name="d8t")
                        nc.sync.dma_start(
                            out=d8[:], in_=dd[:].rearrange("(a b) -> a b", a=128))
                        r8 = bcp.tile([128, 8], f32, tag="r8", name="r8t")
                        nc.vector.reciprocal(r8[:], d8[:])
                        rr = drp.tile([1024], f32, tag="rr", name="rrt")
                        nc.sync.dma_start(out=rr[:].rearrange("(a b) -> a b", a=128),
                                          in_=r8[:])
                        rc = bcp.tile([1, 1024], f32, tag="rc", name="rct")
                        nc.sync.dma_start(out=rc[0:1, :],
                                          in_=rr[:].rearrange("(q b) -> q b", q=1))
                        bc = bcp.tile([64, 1024], f32, tag="bc", name="bct")
                        nc.gpsimd.partition_broadcast(bc[:], rc[:])
                        nc.vector.tensor_tensor(
                            ysb[m][rs, 1024 * jp:1024 * (jp + 1)],
                            yt[:], bc[:], ALU.mult)

            sab.close()
            # ---- S4: out[t, o] ----
            with ExitStack() as s4:
                ps4 = s4.enter_context(tc.tile_pool(name="ps4", bufs=4, space="PSUM"))
                ob = s4.enter_context(tc.tile_pool(name="ob", bufs=4))
                for t in range(NTT):
                    for oc in range(D_MODEL // 512):
                        ps = ps4.tile([128, 512], f32, tag="ps", name="ps4t")
                        for k in range(4):
                            nc.tensor.matmul(
                                ps[:], ysb[k][:, t * 128:(t + 1) * 128],
                                wp[k][:, oc * 512:(oc + 1) * 512],
                                start=(k == 0), stop=(k == 3))
                        o_ = ob.tile([128, 512], f32, tag="o", name="obt")
                        nc.vector.tensor_copy(o_[:], ps[:])
                        nc.sync.dma_start(
                            out=outp[t * 128:(t + 1) * 128, oc * 512:(oc + 1) * 512],
                            in_=o_[:])

    nc.compile()
    return nc


def _prep_core_inputs(x, w_qkv, w_proj, c):
    b, g = c // 2, c % 2
    scale = np.float32(D_HEAD ** -0.5)
    wq = (w_qkv[g * HD:(g + 1) * HD] * scale).astype(np.float32)
    wk = w_qkv[D_MODEL + g * HD:D_MODEL + (g + 1) * HD]
    wv = w_qkv[2 * D_MODEL + g * HD:2 * D_MODEL + (g + 1) * HD]
    tri = np.triu(np.ones((128, 128), dtype=np.float32))
    return {
        "xT": np.ascontiguousarray(x[b].T),
        "wqkT": np.ascontiguousarray(np.concatenate([wq, wk], 0).T),
        "wvT": np.ascontiguousarray(wv.T),
        "wpT": np.ascontiguousarray(w_proj[:, g * HD:(g + 1) * HD].T),
        "trimask": tri,
        "vones": np.ones((128, HPC), dtype=np.float32),
    }


def kernel(x, w_qkv, w_proj):
    x = np.asarray(x)
    w_qkv = np.asarray(w_qkv)
    w_proj = np.asarray(w_proj)
    if "nc" not in _cache:
        _cache["nc"] = _build()
    nc = _cache["nc"]
    in_maps = [_prep_core_inputs(x, w_qkv, w_proj, c) for c in range(N_CORES)]
    res = run_bass_kernel_spmd(nc, in_maps, core_ids=list(range(N_CORES)))
    outs = [res.results[c]["out"] for c in range(N_CORES)]
    return np.stack([outs[2 * b] + outs[2 * b + 1] for b in range(B)], 0)


# revision 22
# speedup vs baseline: 1.3986x; 1.0055x over previous
import sys
import numpy as np

sys.path.insert(0, '/opt/trn_rl_repo')

import concourse.bacc as bacc
import concourse.mybir as mybir
from concourse.bass_utils import run_bass_kernel_spmd
from concourse.tile import TileContext
from contextlib import ExitStack

f32 = mybir.dt.float32
f32r = mybir.dt.float32r
AF = mybir.ActivationFunctionType
ALU = mybir.AluOpType

D_MODEL = 1024
N_HEAD = 16
D_HEAD = 64
B = 4
T = 2048
N_CORES = 8
HPC = N_HEAD // 2        # 8 heads per core
HD = HPC * D_HEAD        # 512 head-dims per core
NTK = D_MODEL // 128     # 8 k-chunks over model dim
NTT = T // 128           # 16 T-tiles of 128
NJP = T // 1024          # 2 col-pairs of 1024

_cache = {}


def _build():
    nc = bacc.Bacc()
    xT = nc.declare_dram_parameter("xT", [D_MODEL, T], f32r, isOutput=False)
    wqkT = nc.declare_dram_parameter("wqkT", [D_MODEL, 2 * HD], f32r, isOutput=False)
    wvT = nc.declare_dram_parameter("wvT", [D_MODEL, HD], f32r, isOutput=False)
    wpT = nc.declare_dram_parameter("wpT", [HD, D_MODEL], f32r, isOutput=False)
    trimask = nc.declare_dram_parameter("trimask", [128, 128], f32r, isOutput=False)
    vones = nc.declare_dram_parameter("vones", [128, HPC], f32r, isOutput=False)
    outp = nc.declare_dram_parameter("out", [T, D_MODEL], f32, isOutput=True)

    with TileContext(nc) as tc, ExitStack() as outer:
        # persistent pools (whole kernel)
        qkp = outer.enter_context(tc.tile_pool(name="qk", bufs=1))
        vp = outer.enter_context(tc.tile_pool(name="v", bufs=1))
        smp = outer.enter_context(tc.tile_pool(name="small", bufs=1))

        qk = [qkp.tile([128, T], f32r, tag=f"qk{m}", name=f"qk{m}") for m in range(8)]
        vt = [None] * NTT
        mask = smp.tile([128, 128], f32r)
        warm = smp.tile([2, 128], f32r)

        # ---- S1: qkT[o,t], v_aug[t, 8*(64+1)]; x streamed in T-halves ----
        with ExitStack() as s1:
            xp = s1.enter_context(tc.tile_pool(name="x", bufs=2))
            wvp = s1.enter_context(tc.tile_pool(name="wv", bufs=1))
            wqp = s1.enter_context(tc.tile_pool(name="wqk", bufs=2))
            ps1 = s1.enter_context(tc.tile_pool(name="ps1", bufs=4, space="PSUM"))
            ps2 = s1.enter_context(tc.tile_pool(name="ps2", bufs=4, space="PSUM"))
            # x first: the first S1a matmuls gate on it; j-split halves so
            # the m=0,j=0 chain starts after ~2MB; ACT HWDGE queue keeps this
            # traffic off the sync queue
            xts = {0: [], 1: []}
            for th in range(2):
                for k in range(NTK):
                    t_ = xp.tile([128, 1024], f32r, tag=f"x{k}", name=f"x{k}_{th}")
                    xts[th].append(t_)
            for th in range(2):
                for jh in range(2):
                    for k in range(NTK):
                        nc.scalar.dma_start(
                            out=xts[th][k][:, jh * 512:(jh + 1) * 512],
                            in_=xT[k * 128:(k + 1) * 128,
                                   1024 * th + jh * 512:1024 * th + (jh + 1) * 512])
            wv = []
            for k in range(NTK):
                t_ = wvp.tile([128, HD], f32r, tag=f"wv{k}", name=f"wv{k}")
                nc.scalar.dma_start(out=t_[:], in_=wvT[k * 128:(k + 1) * 128, :])
                wv.append(t_)
            nc.sync.dma_start(out=mask[:], in_=trimask[:, :])
            # warm up the GPSIMD custom-op library load (~70us) during S1
            nc.gpsimd.partition_broadcast(warm[:], mask[0:1, :])
            for th in range(2):
                hb = 1024 * th
                xt = xts[th]
                # S1a: q,k transposed
                for m in range(8):
                    wqm = []
                    for k in range(NTK):
                        t_ = wqp.tile([128, 128], f32r, tag=f"wm{k}", name=f"wm{k}_{m}_{th}")
                        nc.gpsimd.dma_start(out=t_[:], in_=wqkT[k * 128:(k + 1) * 128,
                                                               m * 128:(m + 1) * 128])
                        wqm.append(t_)
                    for j in range(2):
                        ps = ps1.tile([128, 512], f32, tag="ps", name="ps1t")
                        for k in range(NTK):
                            nc.tensor.matmul(ps[:], wqm[k][:],
                                             xt[k][:, j * 512:(j + 1) * 512],
                                             start=(k == 0), stop=(k == NTK - 1))
                        nc.vector.tensor_copy(qk[m][:, hb + j * 512:hb + (j + 1) * 512], ps[:])
                # S1b: v natural + ones col
                for tl in range(8):
                    t = 8 * th + tl
                    va = vp.tile([128, HPC * 65], f32r, tag=f"v{t}", name=f"v{t}")
                    va3 = va[:].rearrange("p (h e) -> p h e", e=65)
                    nc.gpsimd.dma_start(out=va3[:, :, 64], in_=vones[:, :])
                    ps = ps2.tile([128, HD], f32, tag="psv", name="ps2t")
                    for k in range(NTK):
                        nc.tensor.matmul(ps[:], xt[k][:, tl * 128:(tl + 1) * 128],
                                         wv[k][:, :],
                                         start=(k == 0), stop=(k == NTK - 1))
                    nc.vector.tensor_copy(
                        va3[:, :, 0:64],
                        ps[:].rearrange("p (h e) -> p h e", e=64))
                    vt[t] = va

        # ---- S2/S3: attention, head pairs packed via tile_position ----
        with ExitStack() as sa:
            ysbp = sa.enter_context(tc.tile_pool(name="ysb", bufs=1))
            wpp = sa.enter_context(tc.tile_pool(name="wp", bufs=1))
            ysb = [ysbp.tile([128, T], f32r, tag=f"y{k}", name=f"ysb{k}") for k in range(4)]
            # wp is only read by S4; load it during attention
            wp = []
            for k in range(HD // 128):
                t_ = wpp.tile([128, D_MODEL], f32r, tag=f"wp{k}", name=f"wpt{k}")
                nc.sync.dma_start(out=t_[:], in_=wpT[k * 128:(k + 1) * 128, :])
                wp.append(t_)
            sab = sa.enter_context(ExitStack())
            pp = sab.enter_context(tc.tile_pool(name="p", bufs=6))
            ytp = sab.enter_context(tc.tile_pool(name="yt", bufs=2))
            bcp = sab.enter_context(tc.tile_pool(name="bc", bufs=2))
            drp = sab.enter_context(tc.tile_pool(name="dr", bufs=2, space="DRAM"))
            psA = sab.enter_context(tc.tile_pool(name="psA", bufs=2, space="PSUM"))
            psY = sab.enter_context(tc.tile_pool(name="psY", bufs=1, space="PSUM"))
            for m in range(4):
                hA, hB = 2 * m, 2 * m + 1
                qt, kt = qk[m], qk[4 + m]
                for jp in range(NJP):
                    imax = min(NTT, 8 * jp + 8)
                    psyA = psY.tile([65, 1024], f32, tag="psyA", name="psyAt")
                    psyB = psY.tile([65, 1024], f32, tag="psyB", name="psyBt")
                    psas = {}

                    def emit_qk(i, jp=jp, psas=psas, kt=kt, qt=qt):
                        off = max(0, 128 * i - 1024 * jp)
                        for u in range(2):
                            su = max(off - 512 * u, 0)
                            if su >= 512:
                                continue
                            n = 512 - su
                            tcol = 1024 * jp + 512 * u + su
                            psa = psA.tile([128, 1024], f32, tag="psa", name="psat")
                            nc.tensor.matmul(
                                psa[:, su:512], kt[0:64, i * 128:(i + 1) * 128],
                                qt[0:64, tcol:tcol + n],
                                start=True, stop=True, tile_position=(0, 0))
                            nc.tensor.matmul(
                                psa[:, 512 + su:1024], kt[64:128, i * 128:(i + 1) * 128],
                                qt[64:128, tcol:tcol + n],
                                start=True, stop=True, tile_position=(64, 0))
                            psas[(i, u)] = psa

                    emit_qk(0)
                    for i in range(imax):
                        off = max(0, 128 * i - 1024 * jp)
                        if i + 1 < imax:
                            emit_qk(i + 1)
                        for u in range(2):
                            su = max(off - 512 * u, 0)
                            if su >= 512:
                                continue
                            psa = psas.pop((i, u))
                            pt = pp.tile([128, 1024], f32r, tag="p", name="ptile")
                            p3i = psa[:].rearrange("p (g c) -> p g c", g=2)
                            p3o = pt[:].rearrange("p (g c) -> p g c", g=2)
                            nc.scalar.activation(p3o[:, :, su:512], p3i[:, :, su:512],
                                                 AF.Exp)
                            if 8 * jp <= i and 512 * u <= off < 512 * (u + 1):
                                nc.vector.tensor_tensor(
                                    pt[:, su:su + 128], pt[:, su:su + 128],
                                    mask[:], ALU.mult)
                                nc.vector.tensor_tensor(
                                    pt[:, 512 + su:512 + su + 128],
                                    pt[:, 512 + su:512 + su + 128],
                                    mask[:], ALU.mult)
                            lasti = min(imax, 8 * jp + 4 * (u + 1)) - 1
                            yc = slice(512 * u + su, 512 * u + 512)
                            nc.tensor.matmul(
                                psyA[:, yc], vt[i][:, 65 * hA:65 * hA + 65],
                                pt[:, su:512], start=(i == 0), stop=(i == lasti))
                            nc.tensor.matmul(
                                psyB[:, yc], vt[i][:, 65 * hB:65 * hB + 65],
                                pt[:, 512 + su:1024], start=(i == 0), stop=(i == lasti))
                    for h, psy in ((hA, psyA), (hB, psyB)):
                        rs = slice(64 * (h % 2), 64 * (h % 2) + 64)
                        # evict unnormalized y early so the PSUM bank frees
                        yt = ytp.tile([64, 1024], f32, tag="yt", name="ytt")
                        nc.vector.tensor_copy(yt[:], psy[0:64, :])
                        # reciprocal of denominator row, reshaped across 128
                        # partitions (a [1,1024] DVE op runs on one lane)
                        drow = bcp.tile([1, 1024], f32, tag="drow", name="drowt")
                        nc.scalar.activation(drow[:], psy[64:65, :], AF.Copy)
                        dd = drp.tile([1024], f32, tag="dd", name="ddt")
                        nc.sync.dma_start(out=dd[:], in_=drow[0:1, :])
                        d8 = bcp.tile([128, 8], f32, tag="d8", name="d8t")
                        nc.sync.dma_start(
                            out=d8[:], in_=dd[:].rearrange("(a b) -> a b", a=128))
                        r8 = bcp.tile([128, 8], f32, tag="r8", name="r8t")
                        nc.vector.reciprocal(r8[:], d8[:])
                        rr = drp.tile([1024], f32, tag="rr", name="rrt")
                        nc.sync.dma_start(out=rr[:].rearrange("(a b) -> a b", a=128),
                                          in_=r8[:])
                        rc = bcp.tile([1, 1024], f32, tag="rc", name="rct")
                        nc.sync.dma_start(out=rc[0:1, :],
                                          in_=rr[:].rearrange("(q b) -> q b", q=1))
                        bc = bcp.tile([64, 1024], f32, tag="bc", name="bct")
                        nc.gpsimd.partition_broadcast(bc[:], rc[:])
                        nc.vector.tensor_tensor(
                            ysb[m][rs, 1024 * jp:1024 * (jp + 1)],
                            yt[:], bc[:], ALU.mult)

            sab.close()
            # ---- S4: out[t, o] ----
            with ExitStack() as s4:
                ps4 = s4.enter_context(tc.tile_pool(name="ps4", bufs=4, space="PSUM"))
                ob = s4.enter_context(tc.tile_pool(name="ob", bufs=4))
                for t in range(NTT):
                    for oc in range(D_MODEL // 512):
                        ps = ps4.tile([128, 512], f32, tag="ps", name="ps4t")
                        for k in range(4):
                            nc.tensor.matmul(
                                ps[:], ysb[k][:, t * 128:(t + 1) * 128],
                                wp[k][:, oc * 512:(oc + 1) * 512],
                                start=(k == 0), stop=(k == 3))
                        o_ = ob.tile([128, 512], f32, tag="o", name="obt")
                        nc.vector.tensor_copy(o_[:], ps[:])
                        nc.scalar.dma_start(
                            out=outp[t * 128:(t + 1) * 128, oc * 512:(oc + 1) * 512],
                            in_=o_[:])

    nc.compile()
    return nc


def _prep_core_inputs(x, w_qkv, w_proj, c):
    b, g = c // 2, c % 2
    scale = np.float32(D_HEAD ** -0.5)
    wq = (w_qkv[g * HD:(g + 1) * HD] * scale).astype(np.float32)
    wk = w_qkv[D_MODEL + g * HD:D_MODEL + (g + 1) * HD]
    wv = w_qkv[2 * D_MODEL + g * HD:2 * D_MODEL + (g + 1) * HD]
    tri = np.triu(np.ones((128, 128), dtype=np.float32))
    return {
        "xT": np.ascontiguousarray(x[b].T),
        "wqkT": np.ascontiguousarray(np.concatenate([wq, wk], 0).T),
        "wvT": np.ascontiguousarray(wv.T),
        "wpT": np.ascontiguousarray(w_proj[:, g * HD:(g + 1) * HD].T),
        "trimask": tri,
        "vones": np.ones((128, HPC), dtype=np.float32),
    }


def kernel(x, w_qkv, w_proj):
    x = np.asarray(x)
    w_qkv = np.asarray(w_qkv)
    w_proj = np.asarray(w_proj)
    if "nc" not in _cache:
        _cache["nc"] = _build()
    nc = _cache["nc"]
    in_maps = [_prep_core_inputs(x, w_qkv, w_proj, c) for c in range(N_CORES)]
    res = run_bass_kernel_spmd(nc, in_maps, core_ids=list(range(N_CORES)))
    outs = [res.results[c]["out"] for c in range(N_CORES)]
    return np.stack([outs[2 * b] + outs[2 * b + 1] for b in range(B)], 0)


# revision 23
# speedup vs baseline: 1.4293x; 1.0220x over previous
import sys
import numpy as np

sys.path.insert(0, '/opt/trn_rl_repo')

import concourse.bacc as bacc
import concourse.mybir as mybir
from concourse.bass_utils import run_bass_kernel_spmd
from concourse.tile import TileContext
from contextlib import ExitStack

f32 = mybir.dt.float32
f32r = mybir.dt.float32r
AF = mybir.ActivationFunctionType
ALU = mybir.AluOpType

D_MODEL = 1024
N_HEAD = 16
D_HEAD = 64
B = 4
T = 2048
N_CORES = 8
HPC = N_HEAD // 2        # 8 heads per core
HD = HPC * D_HEAD        # 512 head-dims per core
NTK = D_MODEL // 128     # 8 k-chunks over model dim
NTT = T // 128           # 16 T-tiles of 128
NJP = T // 1024          # 2 col-pairs of 1024

_cache = {}


def _build():
    nc = bacc.Bacc()
    xT = nc.declare_dram_parameter("xT", [D_MODEL, T], f32r, isOutput=False)
    wqkT = nc.declare_dram_parameter("wqkT", [D_MODEL, 2 * HD], f32r, isOutput=False)
    wvT = nc.declare_dram_parameter("wvT", [D_MODEL, HD], f32r, isOutput=False)
    wpT = nc.declare_dram_parameter("wpT", [HD, D_MODEL], f32r, isOutput=False)
    trimask = nc.declare_dram_parameter("trimask", [128, 128], f32r, isOutput=False)
    vones = nc.declare_dram_parameter("vones", [128, HPC], f32r, isOutput=False)
    outp = nc.declare_dram_parameter("out", [T, D_MODEL], f32, isOutput=True)

    with TileContext(nc) as tc, ExitStack() as outer:
        # persistent pools (whole kernel)
        qkp = outer.enter_context(tc.tile_pool(name="qk", bufs=1))
        vp = outer.enter_context(tc.tile_pool(name="v", bufs=1))
        smp = outer.enter_context(tc.tile_pool(name="small", bufs=1))

        qk = [qkp.tile([128, T], f32r, tag=f"qk{m}", name=f"qk{m}") for m in range(8)]
        vt = [None] * NTT
        mask = smp.tile([128, 128], f32r)
        warm = smp.tile([2, 128], f32r)

        # ---- S1: qkT[o,t], v_aug[t, 8*(64+1)]; x streamed in T-halves ----
        with ExitStack() as s1:
            xp = s1.enter_context(tc.tile_pool(name="x", bufs=2))
            wvp = s1.enter_context(tc.tile_pool(name="wv", bufs=1))
            wqp = s1.enter_context(tc.tile_pool(name="wqk", bufs=4))
            ps1 = s1.enter_context(tc.tile_pool(name="ps1", bufs=4, space="PSUM"))
            ps2 = s1.enter_context(tc.tile_pool(name="ps2", bufs=4, space="PSUM"))
            # x first: the first S1a matmuls gate on it; j-split halves so
            # the m=0,j=0 chain starts after ~2MB; ACT HWDGE queue keeps this
            # traffic off the sync queue
            xts = {0: [], 1: []}
            for th in range(2):
                for k in range(NTK):
                    t_ = xp.tile([128, 1024], f32r, tag=f"x{k}", name=f"x{k}_{th}")
                    xts[th].append(t_)
            wv = []
            for th in range(2):
                for jh in range(2):
                    for k in range(NTK):
                        nc.scalar.dma_start(
                            out=xts[th][k][:, jh * 512:(jh + 1) * 512],
                            in_=xT[k * 128:(k + 1) * 128,
                                   1024 * th + jh * 512:1024 * th + (jh + 1) * 512])
                if th == 0:
                    for k in range(NTK):
                        t_ = wvp.tile([128, HD], f32r, tag=f"wv{k}", name=f"wv{k}")
                        nc.scalar.dma_start(out=t_[:], in_=wvT[k * 128:(k + 1) * 128, :])
                        wv.append(t_)
            nc.sync.dma_start(out=mask[:], in_=trimask[:, :])
            # warm up the GPSIMD custom-op library load (~70us) during S1
            nc.gpsimd.partition_broadcast(warm[:], mask[0:1, :])
            for th in range(2):
                hb = 1024 * th
                xt = xts[th]
                # S1a: q,k transposed
                for m in range(8):
                    wqm = []
                    for k in range(NTK):
                        t_ = wqp.tile([128, 128], f32r, tag=f"wm{k}", name=f"wm{k}_{m}_{th}")
                        nc.gpsimd.dma_start(out=t_[:], in_=wqkT[k * 128:(k + 1) * 128,
                                                               m * 128:(m + 1) * 128])
                        wqm.append(t_)
                    for j in range(2):
                        ps = ps1.tile([128, 512], f32, tag="ps", name="ps1t")
                        for k in range(NTK):
                            nc.tensor.matmul(ps[:], wqm[k][:],
                                             xt[k][:, j * 512:(j + 1) * 512],
                                             start=(k == 0), stop=(k == NTK - 1))
                        nc.vector.tensor_copy(qk[m][:, hb + j * 512:hb + (j + 1) * 512], ps[:])
                # S1b: v natural + ones col
                for tl in range(8):
                    t = 8 * th + tl
                    va = vp.tile([128, HPC * 65], f32r, tag=f"v{t}", name=f"v{t}")
                    va3 = va[:].rearrange("p (h e) -> p h e", e=65)
                    nc.gpsimd.dma_start(out=va3[:, :, 64], in_=vones[:, :])
                    ps = ps2.tile([128, HD], f32, tag="psv", name="ps2t")
                    for k in range(NTK):
                        nc.tensor.matmul(ps[:], xt[k][:, tl * 128:(tl + 1) * 128],
                                         wv[k][:, :],
                                         start=(k == 0), stop=(k == NTK - 1))
                    nc.vector.tensor_copy(
                        va3[:, :, 0:64],
                        ps[:].rearrange("p (h e) -> p h e", e=64))
                    vt[t] = va

        # ---- S2/S3: attention, head pairs packed via tile_position ----
        with ExitStack() as sa:
            ysbp = sa.enter_context(tc.tile_pool(name="ysb", bufs=1))
            wpp = sa.enter_context(tc.tile_pool(name="wp", bufs=1))
            ysb = [ysbp.tile([128, T], f32r, tag=f"y{k}", name=f"ysb{k}") for k in range(4)]
            # wp is only read by S4; load it during attention
            wp = []
            for k in range(HD // 128):
                t_ = wpp.tile([128, D_MODEL], f32r, tag=f"wp{k}", name=f"wpt{k}")
                nc.sync.dma_start(out=t_[:], in_=wpT[k * 128:(k + 1) * 128, :])
                wp.append(t_)
            sab = sa.enter_context(ExitStack())
            pp = sab.enter_context(tc.tile_pool(name="p", bufs=6))
            ytp = sab.enter_context(tc.tile_pool(name="yt", bufs=2))
            bcp = sab.enter_context(tc.tile_pool(name="bc", bufs=2))
            drp = sab.enter_context(tc.tile_pool(name="dr", bufs=2, space="DRAM"))
            psA = sab.enter_context(tc.tile_pool(name="psA", bufs=2, space="PSUM"))
            psY = sab.enter_context(tc.tile_pool(name="psY", bufs=1, space="PSUM"))
            for m in range(4):
                hA, hB = 2 * m, 2 * m + 1
                qt, kt = qk[m], qk[4 + m]
                for jp in range(NJP):
                    imax = min(NTT, 8 * jp + 8)
                    psyA = psY.tile([65, 1024], f32, tag="psyA", name="psyAt")
                    psyB = psY.tile([65, 1024], f32, tag="psyB", name="psyBt")
                    psas = {}

                    def emit_qk(i, jp=jp, psas=psas, kt=kt, qt=qt):
                        off = max(0, 128 * i - 1024 * jp)
                        for u in range(2):
                            su = max(off - 512 * u, 0)
                            if su >= 512:
                                continue
                            n = 512 - su
                            tcol = 1024 * jp + 512 * u + su
                            psa = psA.tile([128, 1024], f32, tag="psa", name="psat")
                            nc.tensor.matmul(
                                psa[:, su:512], kt[0:64, i * 128:(i + 1) * 128],
                                qt[0:64, tcol:tcol + n],
                                start=True, stop=True, tile_position=(0, 0))
                            nc.tensor.matmul(
                                psa[:, 512 + su:1024], kt[64:128, i * 128:(i + 1) * 128],
                                qt[64:128, tcol:tcol + n],
                                start=True, stop=True, tile_position=(64, 0))
                            psas[(i, u)] = psa

                    emit_qk(0)
                    for i in range(imax):
                        off = max(0, 128 * i - 1024 * jp)
                        if i + 1 < imax:
                            emit_qk(i + 1)
                        for u in range(2):
                            su = max(off - 512 * u, 0)
                            if su >= 512:
                                continue
                            psa = psas.pop((i, u))
                            pt = pp.tile([128, 1024], f32r, tag="p", name="ptile")
                            p3i = psa[:].rearrange("p (g c) -> p g c", g=2)
                            p3o = pt[:].rearrange("p (g c) -> p g c", g=2)
                            nc.scalar.activation(p3o[:, :, su:512], p3i[:, :, su:512],
                                                 AF.Exp)
                            if 8 * jp <= i and 512 * u <= off < 512 * (u + 1):
                                nc.vector.tensor_tensor(
                                    pt[:, su:su + 128], pt[:, su:su + 128],
                                    mask[:], ALU.mult)
                                nc.vector.tensor_tensor(
                                    pt[:, 512 + su:512 + su + 128],
                                    pt[:, 512 + su:512 + su + 128],
                                    mask[:], ALU.mult)
                            lasti = min(imax, 8 * jp + 4 * (u + 1)) - 1
                            yc = slice(512 * u + su, 512 * u + 512)
                            nc.tensor.matmul(
                                psyA[:, yc], vt[i][:, 65 * hA:65 * hA + 65],
                                pt[:, su:512], start=(i == 0), stop=(i == lasti))
                            nc.tensor.matmul(
                                psyB[:, yc], vt[i][:, 65 * hB:65 * hB + 65],
                                pt[:, 512 + su:1024], start=(i == 0), stop=(i == lasti))
                    for h, psy in ((hA, psyA), (hB, psyB)):
                        rs = slice(64 * (h % 2), 64 * (h % 2) + 64)
                        # evict unnormalized y early so the PSUM bank frees
                        yt = ytp.tile([64, 1024], f32, tag="yt", name="ytt")
                        nc.vector.tensor_copy(yt[:], psy[0:64, :])
                        # reciprocal of denominator row, reshaped across 128
                        # partitions (a [1,1024] DVE op runs on one lane)
                        drow = bcp.tile([1, 1024], f32, tag="drow", name="drowt")
                        nc.scalar.activation(drow[:], psy[64:65, :], AF.Copy)
                        dd = drp.tile([1024], f32, tag="dd", name="ddt")
                        nc.sync.dma_start(out=dd[:], in_=drow[0:1, :])
                        d8 = bcp.tile([128, 8], f32, tag="d8", name="d8t")
                        nc.sync.dma_start(
                            out=d8[:], in_=dd[:].rearrange("(a b) -> a b", a=128))
                        r8 = bcp.tile([128, 8], f32, tag="r8", name="r8t")
                        nc.vector.reciprocal(r8[:], d8[:])
                        rr = drp.tile([1024], f32, tag="rr", name="rrt")
                        nc.sync.dma_start(out=rr[:].rearrange("(a b) -> a b", a=128),
                                          in_=r8[:])
                        rc = bcp.tile([1, 1024], f32, tag="rc", name="rct")
                        nc.sync.dma_start(out=rc[0:1, :],
                                          in_=rr[:].rearrange("(q b) -> q b", q=1))
                        bc = bcp.tile([64, 1024], f32, tag="bc", name="bct")
                        nc.gpsimd.partition_broadcast(bc[:], rc[:])
                        nc.vector.tensor_tensor(
                            ysb[m][rs, 1024 * jp:1024 * (jp + 1)],
                            yt[:], bc[:], ALU.mult)

            sab.close()
            # ---- S4: out[t, o] ----
            with ExitStack() as s4:
                ps4 = s4.enter_context(tc.tile_pool(name="ps4", bufs=4, space="PSUM"))
                ob = s4.enter_context(tc.tile_pool(name="ob", bufs=4))
                for t in range(NTT):
                    for oc in range(D_MODEL // 512):
                        ps = ps4.tile([128, 512], f32, tag="ps", name="ps4t")
                        for k in range(4):
                            nc.tensor.matmul(
                                ps[:], ysb[k][:, t * 128:(t + 1) * 128],
                                wp[k][:, oc * 512:(oc + 1) * 512],
                                start=(k == 0), stop=(k == 3))
                        o_ = ob.tile([128, 512], f32, tag="o", name="obt")
                        nc.vector.tensor_copy(o_[:], ps[:])
                        nc.scalar.dma_start(
                            out=outp[t * 128:(t + 1) * 128, oc * 512:(oc + 1) * 512],
                            in_=o_[:])

    nc.compile()
    return nc


def _prep_core_inputs(x, w_qkv, w_proj, c):
    b, g = c // 2, c % 2
    scale = np.float32(D_HEAD ** -0.5)
    wq = (w_qkv[g * HD:(g + 1) * HD] * scale).astype(np.float32)
    wk = w_qkv[D_MODEL + g * HD:D_MODEL + (g + 1) * HD]
    wv = w_qkv[2 * D_MODEL + g * HD:2 * D_MODEL + (g + 1) * HD]
    tri = np.triu(np.ones((128, 128), dtype=np.float32))
    return {
        "xT": np.ascontiguousarray(x[b].T),
        "wqkT": np.ascontiguousarray(np.concatenate([wq, wk], 0).T),
        "wvT": np.ascontiguousarray(wv.T),
        "wpT": np.ascontiguousarray(w_proj[:, g * HD:(g + 1) * HD].T),
        "trimask": tri,
        "vones": np.ones((128, HPC), dtype=np.float32),
    }


def kernel(x, w_qkv, w_proj):
    x = np.asarray(x)
    w_qkv = np.asarray(w_qkv)
    w_proj = np.asarray(w_proj)
    if "nc" not in _cache:
        _cache["nc"] = _build()
    nc = _cache["nc"]
    in_maps = [_prep_core_inputs(x, w_qkv, w_proj, c) for c in range(N_CORES)]
    res = run_bass_kernel_spmd(nc, in_maps, core_ids=list(range(N_CORES)))
    outs = [res.results[c]["out"] for c in range(N_CORES)]
    return np.stack([outs[2 * b] + outs[2 * b + 1] for b in range(B)], 0)
